# revision 2
# baseline (speedup 1.0000x reference)
"""VQ codebook assignment (GraphTokenizer.assign_nearest) on 8 TRN2 NeuronCores.

Math: for each embedding row x, find argmin_m ||x - c_m||^2 and gather c_m.
argmin_m ||x-c||^2 == argmax_m (2x.c - ||c||^2)  (||x||^2 constant per row).

Sharding: data-parallel over the node axis N (65536 -> 8 x 8192); codebook
replicated.  Per core the PE computes G = (2X) @ C^T - csq via three
full-rate bf16 matmul passes (hi/lo split of both operands; the dropped
lo*lo term is ~2^-18 relative) accumulated in fp32 PSUM, plus one K=3
matmul adding -||c||^2 as an exact 3-part bf16 triplet.  ACT drains PSUM,
DVE max/max_index produce the argmax, GPSIMD indirect-DMA gathers the
winning codebook rows.
"""

import numpy as np
import ml_dtypes
from contextlib import ExitStack

import concourse.bass as bass
import concourse.tile as tile
from concourse import bacc, mybir
from concourse.bass_utils import run_bass_kernel_spmd

P = 128
D = 512
M = 4096
N_FULL = 65536
N_CORES = 8
NS = N_FULL // N_CORES  # 8192 rows per core
KC = D // P             # 4 contraction chunks
MH = 2048               # m-half (4 PSUM banks)
BF16 = mybir.dt.bfloat16
F32 = mybir.dt.float32
U32 = mybir.dt.uint32


def build_nc(ns=NS, super_n=1024):
    """Build the single-core Bass program (run SPMD on 8 cores)."""
    assert ns % P == 0 and super_n % P == 0 and ns % super_n == 0
    nc = bacc.Bacc(None, target_bir_lowering=False)

    xt_hi = nc.declare_dram_parameter("xt_hi", [D, ns], BF16, isOutput=False)
    xt_lo = nc.declare_dram_parameter("xt_lo", [D, ns], BF16, isOutput=False)
    cbt_hi = nc.declare_dram_parameter("cbt_hi", [D, M], BF16, isOutput=False)
    cbt_lo = nc.declare_dram_parameter("cbt_lo", [D, M], BF16, isOutput=False)
    cbt_f32 = nc.declare_dram_parameter("cbt_f32", [D, M], F32, isOutput=False)
    cb_rows = nc.declare_dram_parameter("cb_rows", [M, D], F32, isOutput=False)
    quant = nc.declare_dram_parameter("quant", [ns, D], F32, isOutput=True)
    idx_out = nc.declare_dram_parameter("idx_out", [ns], U32, isOutput=True)

    bps = super_n // P  # blocks per super-block

    with tile.TileContext(nc) as tc:
        with ExitStack() as stk:
            const = stk.enter_context(tc.tile_pool(name="const", bufs=1))

            cbh, cbl = [], []
            for k in range(KC):
                t = const.tile([P, M], BF16, tag=f"cbh{k}")
                nc.sync.dma_start(t[:], cbt_hi[k * P:(k + 1) * P, :])
                cbh.append(t)
                t = const.tile([P, M], BF16, tag=f"cbl{k}")
                nc.sync.dma_start(t[:], cbt_lo[k * P:(k + 1) * P, :])
                cbl.append(t)
            csq3 = const.tile([3, M], BF16, tag="csq3")
            ones3 = const.tile([3, P], BF16, tag="ones3")
            nc.gpsimd.memset(ones3[:], 1.0)
            ones1 = const.tile([P, 1], F32, tag="ones1")
            nc.gpsimd.memset(ones1[:], 1.0)

            # ---- preamble: csq3 = 3-part bf16 split of -||c||^2 ----
            # NOTE: a [1, M] tile still reserves its column range on all 128
            # partitions, so keep this pool lean (bufs=1, reused scratch).
            with tc.tile_pool(name="pre", bufs=1) as pre, \
                 tc.tile_pool(name="prepsum", bufs=1, space="PSUM") as prepsum:
                csq_ps = prepsum.tile([1, M], F32, tag="csqps")
                rem = pre.tile([1, M], F32, tag="rem")
                pf = pre.tile([1, M], F32, tag="pf")
                for k in range(KC):
                    cf = pre.tile([P, M], F32, tag="cf")
                    nc.sync.dma_start(cf[:], cbt_f32[k * P:(k + 1) * P, :])
                    sq = pre.tile([P, M], F32, tag="sq")
                    nc.scalar.square(sq[:], cf[:])
                    for b in range(M // 512):
                        nc.tensor.matmul(
                            csq_ps[0:1, b * 512:(b + 1) * 512],
                            lhsT=ones1[:],
                            rhs=sq[:, b * 512:(b + 1) * 512],
                            start=(k == 0), stop=(k == KC - 1),
                        )
                nc.scalar.mul(rem[:], csq_ps[0:1, :], -1.0)
                for i in range(3):
                    pb = pre.tile([1, M], BF16, tag="pb")
                    nc.vector.tensor_copy(pb[:], rem[:])
                    # cross-partition move onto row i of csq3 -> DMA
                    nc.sync.dma_start(csq3[i:i + 1, :], pb[:])
                    if i < 2:
                        nc.vector.tensor_copy(pf[:], pb[:])
                        nc.vector.tensor_sub(rem[:], rem[:], pf[:])

            xpool = stk.enter_context(tc.tile_pool(name="x", bufs=2))
            pspool = stk.enter_context(tc.tile_pool(name="ps", bufs=2, space="PSUM"))
            dpool = stk.enter_context(tc.tile_pool(name="dist", bufs=3))
            mpool = stk.enter_context(tc.tile_pool(name="small", bufs=6))
            qpool = stk.enter_context(tc.tile_pool(name="q", bufs=3))

            for sbi in range(ns // super_n):
                xh = xpool.tile([P, KC * super_n], BF16, tag="xh")
                xl = xpool.tile([P, KC * super_n], BF16, tag="xl")
                for k in range(KC):
                    nc.sync.dma_start(
                        xh[:, k * super_n:(k + 1) * super_n],
                        xt_hi[k * P:(k + 1) * P, sbi * super_n:(sbi + 1) * super_n])
                    nc.sync.dma_start(
                        xl[:, k * super_n:(k + 1) * super_n],
                        xt_lo[k * P:(k + 1) * P, sbi * super_n:(sbi + 1) * super_n])

                for j in range(bps):
                    blk = sbi * bps + j
                    n0 = blk * P
                    dist = dpool.tile([P, M], F32, tag="dist")
                    for half in range(2):
                        ps = pspool.tile([P, MH], F32, tag="ps")
                        for b in range(MH // 512):
                            m0 = half * MH + b * 512
                            out_ap = ps[:, b * 512:(b + 1) * 512]
                            first = True
                            for xt, cbt in ((xh, cbh), (xh, cbl), (xl, cbh)):
                                for k in range(KC):
                                    nc.tensor.matmul(
                                        out_ap,
                                        lhsT=xt[:, k * super_n + j * P:
                                                k * super_n + (j + 1) * P],
                                        rhs=cbt[k][:, m0:m0 + 512],
                                        start=first, stop=False)
                                    first = False
                            nc.tensor.matmul(out_ap, lhsT=ones3[:],
                                             rhs=csq3[:, m0:m0 + 512],
                                             start=False, stop=True)
                        nc.scalar.copy(dist[:, half * MH:(half + 1) * MH], ps[:])

                    mx = mpool.tile([P, 8], F32, tag="mx")
                    nc.vector.max(mx[:], dist[:])
                    ix = mpool.tile([P, 8], U32, tag="ix")
                    nc.vector.max_index(ix[:], mx[:], dist[:])

                    q = qpool.tile([P, D], F32, tag="q")
                    nc.gpsimd.indirect_dma_start(
                        out=q[:], out_offset=None,
                        in_=cb_rows[:],
                        in_offset=bass.IndirectOffsetOnAxis(ap=ix[:, 0:1], axis=0),
                    )
                    nc.sync.dma_start(quant[n0:n0 + P, :], q[:])
                    nc.sync.dma_start(idx_out[n0:n0 + P, None], ix[:, 0:1])

    nc.finalize()
    return nc


_NC_CACHE = {}


def _get_nc(ns=NS, super_n=1024):
    key = (ns, super_n)
    if key not in _NC_CACHE:
        _NC_CACHE[key] = build_nc(ns, super_n)
    return _NC_CACHE[key]


def _split_bf16(a):
    hi = a.astype(ml_dtypes.bfloat16)
    lo = (a - hi.astype(np.float32)).astype(ml_dtypes.bfloat16)
    return hi, lo


def make_in_maps(embeddings, codebook, ns=NS, n_cores=N_CORES):
    emb = np.ascontiguousarray(np.asarray(embeddings, dtype=np.float32))
    cb = np.ascontiguousarray(np.asarray(codebook, dtype=np.float32))
    ch, cl = _split_bf16(cb)
    shared = {
        "cbt_hi": np.ascontiguousarray(ch.T),
        "cbt_lo": np.ascontiguousarray(cl.T),
        "cbt_f32": np.ascontiguousarray(cb.T),
        "cb_rows": cb,
    }
    x2 = 2.0 * emb
    xh, xl = _split_bf16(x2)
    in_maps = []
    for s in range(n_cores):
        sl = slice(s * ns, (s + 1) * ns)
        in_maps.append({
            "xt_hi": np.ascontiguousarray(xh[sl].T),
            "xt_lo": np.ascontiguousarray(xl[sl].T),
            **shared,
        })
    return in_maps


def kernel(embeddings, codebook):
    nc = _get_nc()
    in_maps = make_in_maps(embeddings, codebook)
    res = run_bass_kernel_spmd(nc, in_maps, core_ids=list(range(N_CORES)))
    quant = np.concatenate([r["quant"] for r in res.results], axis=0)
    idx = np.concatenate(
        [r["idx_out"].astype(np.int32) for r in res.results], axis=0)
    return quant, idx


# revision 8
# speedup vs baseline: 1.2172x; 1.2172x over previous
"""VQ codebook assignment (GraphTokenizer.assign_nearest) on 8 TRN2 NeuronCores.

Math: for each embedding row x, find argmin_m ||x - c_m||^2 and gather c_m.
argmin_m ||x-c||^2 == argmax_m (2x.c - ||c||^2)  (||x||^2 constant per row).

Sharding: data-parallel over the node axis N (65536 -> 8 x 8192); codebook
replicated.  Per core the PE computes G = (2X) @ C^T - csq via three
full-rate bf16 matmul passes (hi/lo split of both operands; the dropped
lo*lo term is ~2^-18 relative) accumulated in fp32 PSUM, plus one K=3
matmul adding -||c||^2 as an exact 3-part bf16 triplet.  ACT drains PSUM,
DVE max/max_index produce the argmax, GPSIMD indirect-DMA gathers the
winning codebook rows.
"""

import numpy as np
import ml_dtypes
from contextlib import ExitStack

import concourse.bass as bass
import concourse.tile as tile
from concourse import bacc, mybir
from concourse.bass_utils import run_bass_kernel_spmd

P = 128
D = 512
M = 4096
N_FULL = 65536
N_CORES = 8
NS = N_FULL // N_CORES  # 8192 rows per core
KC = D // P             # 4 contraction chunks
MH = 2048               # m-half (4 PSUM banks)
BF16 = mybir.dt.bfloat16
F32 = mybir.dt.float32
U32 = mybir.dt.uint32


def build_nc(ns=NS, super_n=1024):
    """Build the single-core Bass program (run SPMD on 8 cores)."""
    assert ns % P == 0 and super_n % P == 0 and ns % super_n == 0
    nc = bacc.Bacc(None, target_bir_lowering=False)

    xt_hi = nc.declare_dram_parameter("xt_hi", [D, ns], BF16, isOutput=False)
    xt_lo = nc.declare_dram_parameter("xt_lo", [D, ns], BF16, isOutput=False)
    cbt_hi = nc.declare_dram_parameter("cbt_hi", [D, M], BF16, isOutput=False)
    cbt_lo = nc.declare_dram_parameter("cbt_lo", [D, M], BF16, isOutput=False)
    csq_in = nc.declare_dram_parameter("csq_rep", [P, M], F32, isOutput=False)
    cb_rows = nc.declare_dram_parameter("cb_rows", [M, D], F32, isOutput=False)
    quant = nc.declare_dram_parameter("quant", [ns, D], F32, isOutput=True)
    idx_out = nc.declare_dram_parameter("idx_out", [ns], U32, isOutput=True)

    bps = super_n // P  # blocks per super-block

    with tile.TileContext(nc) as tc:
        with ExitStack() as stk:
            const = stk.enter_context(tc.tile_pool(name="const", bufs=1))

            cbh, cbl = [], []
            for k in range(KC):
                t = const.tile([P, M], BF16, tag=f"cbh{k}")
                nc.sync.dma_start(t[:], cbt_hi[k * P:(k + 1) * P, :])
                cbh.append(t)
                t = const.tile([P, M], BF16, tag=f"cbl{k}")
                nc.sync.dma_start(t[:], cbt_lo[k * P:(k + 1) * P, :])
                cbl.append(t)
            csq_rep = const.tile([P, M], F32, tag="csq_rep")
            nc.sync.dma_start(csq_rep[:], csq_in[:])

            xpool = stk.enter_context(tc.tile_pool(name="x", bufs=2))
            pspool = stk.enter_context(tc.tile_pool(name="ps", bufs=2, space="PSUM"))
            dpool = stk.enter_context(tc.tile_pool(name="dist", bufs=3))
            mpool = stk.enter_context(tc.tile_pool(name="small", bufs=6))
            qpool = stk.enter_context(tc.tile_pool(name="q", bufs=3))

            for sbi in range(ns // super_n):
                xh = xpool.tile([P, KC * super_n], BF16, tag="xh")
                xl = xpool.tile([P, KC * super_n], BF16, tag="xl")
                for k in range(KC):
                    nc.sync.dma_start(
                        xh[:, k * super_n:(k + 1) * super_n],
                        xt_hi[k * P:(k + 1) * P, sbi * super_n:(sbi + 1) * super_n])
                    nc.sync.dma_start(
                        xl[:, k * super_n:(k + 1) * super_n],
                        xt_lo[k * P:(k + 1) * P, sbi * super_n:(sbi + 1) * super_n])

                for j in range(bps):
                    blk = sbi * bps + j
                    n0 = blk * P
                    dist = dpool.tile([P, M], F32, tag="dist")
                    for half in range(2):
                        ps = pspool.tile([P, MH], F32, tag="ps")
                        # weight-major: one lhsT serves all 4 banks of the half
                        passes = [(xh, cbh), (xh, cbl), (xl, cbh)]
                        n_w = len(passes) * KC
                        wi = 0
                        for xt, cbt in passes:
                            for k in range(KC):
                                lhsT = xt[:, k * super_n + j * P:
                                          k * super_n + (j + 1) * P]
                                for b in range(MH // 512):
                                    m0 = half * MH + b * 512
                                    nc.tensor.matmul(
                                        ps[:, b * 512:(b + 1) * 512],
                                        lhsT=lhsT,
                                        rhs=cbt[k][:, m0:m0 + 512],
                                        start=(wi == 0), stop=(wi == n_w - 1))
                                wi += 1
                        # drain: dist = cross - csq  (argmax dist == argmin d2)
                        nc.vector.tensor_sub(
                            dist[:, half * MH:(half + 1) * MH], ps[:],
                            csq_rep[:, half * MH:(half + 1) * MH])

                    mx = mpool.tile([P, 8], F32, tag="mx")
                    nc.vector.max(mx[:], dist[:])
                    ix = mpool.tile([P, 8], U32, tag="ix")
                    nc.vector.max_index(ix[:], mx[:], dist[:])

                    q = qpool.tile([P, D], F32, tag="q")
                    nc.gpsimd.indirect_dma_start(
                        out=q[:], out_offset=None,
                        in_=cb_rows[:],
                        in_offset=bass.IndirectOffsetOnAxis(ap=ix[:, 0:1], axis=0),
                    )
                    nc.sync.dma_start(quant[n0:n0 + P, :], q[:])
                    nc.sync.dma_start(idx_out[n0:n0 + P, None], ix[:, 0:1])

    nc.finalize()
    return nc


_NC_CACHE = {}


def _get_nc(ns=NS, super_n=1024):
    key = (ns, super_n)
    if key not in _NC_CACHE:
        _NC_CACHE[key] = build_nc(ns, super_n)
    return _NC_CACHE[key]


def _split_bf16(a):
    hi = a.astype(ml_dtypes.bfloat16)
    lo = (a - hi.astype(np.float32)).astype(ml_dtypes.bfloat16)
    return hi, lo


def make_in_maps(embeddings, codebook, ns=NS, n_cores=N_CORES):
    emb = np.ascontiguousarray(np.asarray(embeddings, dtype=np.float32))
    cb = np.ascontiguousarray(np.asarray(codebook, dtype=np.float32))
    ch, cl = _split_bf16(cb)
    csq = (cb.astype(np.float64) ** 2).sum(axis=1).astype(np.float32)
    shared = {
        "cbt_hi": np.ascontiguousarray(ch.T),
        "cbt_lo": np.ascontiguousarray(cl.T),
        "csq_rep": np.ascontiguousarray(
            np.broadcast_to(csq[None, :], (P, M)).copy()),
        "cb_rows": cb,
    }
    x2 = 2.0 * emb
    xh, xl = _split_bf16(x2)
    in_maps = []
    for s in range(n_cores):
        sl = slice(s * ns, (s + 1) * ns)
        in_maps.append({
            "xt_hi": np.ascontiguousarray(xh[sl].T),
            "xt_lo": np.ascontiguousarray(xl[sl].T),
            **shared,
        })
    return in_maps


def kernel(embeddings, codebook):
    nc = _get_nc()
    in_maps = make_in_maps(embeddings, codebook)
    res = run_bass_kernel_spmd(nc, in_maps, core_ids=list(range(N_CORES)))
    quant = np.concatenate([r["quant"] for r in res.results], axis=0)
    idx = np.concatenate(
        [r["idx_out"].astype(np.int32) for r in res.results], axis=0)
    return quant, idx
